# revision 23
# baseline (speedup 1.0000x reference)
"""Trainium2 Bass kernel for nn_BatchMeshDeformationBlock (batched mesh GCN).

Strategy: data-parallel over batch B=32 across 8 NeuronCores (4 batches/core),
adj + weights replicated. BatchNorm batch statistics are combined with an
AllGather of per-vertex (sum, sumsq) partials + a local 8-way reduce.

Per-core layout:
  - activations flow in "normal" layout tiles [128 v-partitions, 192 ch]
  - feature matmuls consume transposed bf16 chunks produced by DMA-transpose
  - adj^T stays resident in SBUF as bf16 [128, 21 vtiles, 2562]
  - (feats + y)/2 averaging is carried unnormalized (feats_stored = 2^k * feats)
    with the 2^-k folded into the next layer's weights host-side; the final
    /128 is applied on the host.
"""
import os
import sys

os.environ.setdefault("JAX_PLATFORMS", "axon,cpu")
if "/opt/trn_rl_repo" not in sys.path:
    sys.path.insert(0, "/opt/trn_rl_repo")

import numpy as np
import ml_dtypes

import concourse.bass as bass
import concourse.bacc as bacc
import concourse.mybir as mybir
import concourse.tile as tile
from concourse.bass_utils import run_bass_kernel_spmd

BF16NP = ml_dtypes.bfloat16
F32 = mybir.dt.float32
BF = mybir.dt.bfloat16
AL = mybir.AluOpType
AF = mybir.ActivationFunctionType

# ---- problem constants (hardcoded per contract) ----
B, V, H, CIN, PD, OUT = 32, 2562, 192, 966, 963, 3
EPS = 1e-5
NCORE = 8
BC = B // NCORE              # 4 batches per core
NT = (V + 127) // 128        # 21 vertex tiles
VP = NT * 128                # 2688 padded vertices
RT = BC * NT                 # 84 row tiles per core
R = RT * 128                 # 10752 padded rows per core
SL = 64                      # side_len for H=192
K1T = 8                      # K tiles for layer 0 (966 -> 1024)
NW = 34                      # weight slots: 8 + 12*2 + 2
CNT = float(B * H)           # BN count = 6144

# BN layer table: input-scale fold k_in (W_eff = W * 2^-k_in) and
# y-scale for update layers (folded into gamma/beta host-side).
KIN = [0, 0, 1, 0, 2, 0, 3, 0, 4, 0, 5, 0, 6]   # per BN layer 0..12
KIN_FINAL = 7
YSC = {1: 1.0, 3: 2.0, 5: 4.0, 7: 8.0, 9: 16.0, 11: 32.0, 12: 64.0}
UPDATE_LAYERS = set(YSC.keys())   # layers whose y updates feats
N_LAYERS = int(os.environ.get("BMDB_LAYERS", "13"))
WITH_FINAL = os.environ.get("BMDB_SKIP_FINAL", "0") != "1" and N_LAYERS == 13
PHASE = int(os.environ.get("BMDB_PHASE", "9"))   # debug truncation knob


def _bf(x):
    return np.ascontiguousarray(x.astype(np.float32).astype(BF16NP))


def _f32(x):
    return np.ascontiguousarray(np.asarray(x, dtype=np.float32))


def host_prep(inputs):
    """Build the 8 per-core input maps (numpy only)."""
    features = _f32(inputs["features"])
    pooled = _f32(inputs["pooled"])
    adj = _f32(inputs["adj"])
    W1 = _f32(inputs["W1"]); b1 = _f32(inputs["b1"])
    Wm = _f32(inputs["Wm"]); bm = _f32(inputs["bm"])
    W15 = _f32(inputs["W15"]); b15 = _f32(inputs["b15"])
    gamma = _f32(inputs["gamma"]); beta = _f32(inputs["beta"])

    full = np.concatenate([features, pooled], axis=-1)      # [B, V, 966]

    # adj^T resident tiles [128, NT, V], with all-ones bias row at v=VP-1
    adjT = np.zeros((VP, V), np.float32)
    adjT[:V] = adj.T
    adjT[VP - 1] = 1.0
    adjTb = _bf(adjT.reshape(NT, 128, V).transpose(1, 0, 2))  # [128, NT, V]

    # weight slots [NW, 128, 192]
    ws = np.zeros((NW, 128, H), np.float32)
    W1p = np.zeros((K1T * 128, H), np.float32)
    W1p[:CIN] = W1  # k_in = 0
    ws[0:K1T] = W1p.reshape(K1T, 128, H)
    blist = [b1]
    for i in range(12):
        l = i + 1
        Wl = Wm[i] * (2.0 ** -KIN[l])
        ws[K1T + 2 * i][:] = Wl[0:128]
        ws[K1T + 2 * i + 1][0:64] = Wl[128:192]
        blist.append(bm[i])
    W15l = W15 * (2.0 ** -KIN_FINAL)
    ws[32, :, 0:OUT] = W15l[0:128]
    ws[33, 0:64, 0:OUT] = W15l[128:192]
    wsb = _bf(ws)

    # per-layer bias broadcast [13, 128, 128] (channels 64..191) and Y bias rows
    bbc = np.zeros((13, 128, 128), np.float32)
    ybias = np.zeros((13, 4 * SL), np.float32)
    for l in range(13):
        bbc[l, :, :] = blist[l][None, 64:192]
        ybias[l] = np.tile(blist[l][128:192], BC)
    bbcb = _bf(bbc)
    ybiasb = _bf(ybias)
    ybias15 = _bf(np.tile(b15[0:2], BC).reshape(1, 2 * BC))   # [1, 8]

    # recip norm [128, NT] (pad -> 0)
    deg = adj.sum(axis=1)
    rn = np.zeros(VP, np.float32)
    rn[:V] = 1.0 / deg
    rnb = np.ascontiguousarray(rn.reshape(NT, 128).T)   # [128, NT]

    # gamma/beta effective [128, 13, NT, 2]
    gb = np.zeros((VP, 13, 2), np.float32)
    gb[:, :, 0] = 1.0
    for l in range(13):
        s = YSC.get(l, 1.0)
        gb[:V, l, 0] = gamma[l] * s
        gb[:V, l, 1] = beta[l] * s
    gbb = np.ascontiguousarray(
        gb.reshape(NT, 128, 13, 2).transpose(1, 2, 0, 3))  # [128, 13, NT, 2]

    in_maps = []
    for c in range(NCORE):
        fb = full[c * BC:(c + 1) * BC]                      # [4, V, 966]
        x0 = np.zeros((BC, VP, K1T * 128), np.float32)
        x0[:, :V, :CIN] = fb
        x0t = _bf(x0.transpose(2, 0, 1).reshape(K1T * 128, R)
                  .reshape(K1T, 128, R))                    # [8, 128, R]
        f192 = np.zeros((BC, VP, H), np.float32)
        f192[:, :V] = fb[:, :, :H]
        f192 = np.ascontiguousarray(
            f192.reshape(BC * NT, 128, H))                  # [84, 128, 192]
        in_maps.append(dict(
            x0t=x0t, f192=f192, adjt=adjTb, ws=wsb, bbc=bbcb,
            ybias=ybiasb, ybias15=ybias15, rn=rnb, gb=gbb,
        ))
    meta = dict(b15_2=float(b15[2]))
    return in_maps, meta


def build_program(meta):
    nc = bacc.Bacc("TRN2", target_bir_lowering=False, debug=False,
                   num_devices=NCORE)
    # ---- I/O ----
    x0t_in = nc.dram_tensor("x0t", [K1T, 128, R], BF, kind="ExternalInput")
    f192_in = nc.dram_tensor("f192", [RT, 128, H], F32, kind="ExternalInput")
    adjt_in = nc.dram_tensor("adjt", [128, NT, V], BF, kind="ExternalInput")
    ws_in = nc.dram_tensor("ws", [NW, 128, H], BF, kind="ExternalInput")
    bbc_in = nc.dram_tensor("bbc", [13, 128, 128], BF, kind="ExternalInput")
    ybias_in = nc.dram_tensor("ybias", [13, 4 * SL], BF, kind="ExternalInput")
    ybias15_in = nc.dram_tensor("ybias15", [1, 2 * BC], BF, kind="ExternalInput")
    rn_in = nc.dram_tensor("rn", [128, NT], F32, kind="ExternalInput")
    gb_in = nc.dram_tensor("gb", [128, 13, NT, 2], F32, kind="ExternalInput")

    feats_out = nc.dram_tensor("feats_raw", [RT, 128, H], F32,
                               kind="ExternalOutput")
    c2_out = nc.dram_tensor("c2", [128, RT], F32, kind="ExternalOutput")
    st15_out = nc.dram_tensor("st15", [2 * BC, VP], F32, kind="ExternalOutput")
    debug = N_LAYERS < 13
    if debug:
        dbg_xp = nc.dram_tensor("dbg_xp", [128, RT, H], BF,
                                kind="ExternalOutput")
        dbg_y = nc.dram_tensor("dbg_y", [RT, 128, H], BF,
                               kind="ExternalOutput")

    # collective bounce buffers (internal DRAM)
    ag_in = [nc.dram_tensor(f"agin{l}", [128, NT * 2], F32)
             for l in range(N_LAYERS)]
    ag_out = [nc.dram_tensor(f"agout{l}", [NCORE, 128, NT * 2], F32)
              for l in range(N_LAYERS)]
    warm_in = nc.dram_tensor("warmin", [128, 2], F32)
    warm_out = nc.dram_tensor("warmout", [NCORE, 128, 2], F32)

    rg = [list(range(NCORE))]

    with tile.TileContext(nc) as tc:
        with tc.tile_pool(name="persist", bufs=1) as persist, \
             tc.tile_pool(name="chunks", bufs=8) as chpool, \
             tc.tile_pool(name="ytile", bufs=4) as ypool, \
             tc.tile_pool(name="ftile", bufs=3) as fpool, \
             tc.tile_pool(name="x0", bufs=2) as x0pool, \
             tc.tile_pool(name="scr", bufs=2) as scrpool, \
             tc.tile_pool(name="stat", bufs=2) as stpool, \
             tc.tile_pool(name="psA", bufs=3, space="PSUM") as psA, \
             tc.tile_pool(name="psB", bufs=3, space="PSUM") as psB:

            # ---- persistent tensors ----
            adjT = persist.tile([128, NT, V], BF, tag="adjT")
            Y = persist.tile([128, NT, BC * SL], BF, tag="Y")
            xp = persist.tile([128, RT, H], BF, tag="xp")
            ws = persist.tile([128, NW, H], BF, tag="ws")
            bbc = persist.tile([128, 13, 128], BF, tag="bbc")
            gb = persist.tile([128, 13, NT, 2], F32, tag="gb")
            rn = persist.tile([128, NT], F32, tag="rn")
            sA = persist.tile([128, RT], F32, tag="sA")
            sB = persist.tile([128, RT], F32, tag="sB")
            sq = persist.tile([128, RT], F32, tag="sq")
            c2sb = persist.tile([128, RT], F32, tag="c2sb")
            y15 = persist.tile([128, NT, 2 * BC], BF, tag="y15")
            eps_t = persist.tile([128, 1], F32, tag="eps")
            c2b_t = persist.tile([128, 1], F32, tag="c2b")
            nc.vector.memset(eps_t[:], EPS)
            nc.vector.memset(c2b_t[:], meta["b15_2"])

            # ---- init loads ----
            nc.sync.dma_start(out=adjT[:], in_=adjt_in[:])
            nc.sync.dma_start(out=ws[:], in_=ws_in[:].rearrange("s p h -> p s h"))
            nc.sync.dma_start(out=bbc[:], in_=bbc_in[:].rearrange("l p c -> p l c"))
            nc.sync.dma_start(out=gb[:], in_=gb_in[:])
            nc.sync.dma_start(out=rn[:], in_=rn_in[:])

            # init memsets (avoid NaN garbage in stale/pad regions)
            nc.gpsimd.memset(xp[:], 0.0)
            nc.gpsimd.memset(Y[:], 0.0)
            nc.vector.memset(sA[:], 0.0)
            nc.vector.memset(sB[:], 0.0)
            nc.vector.memset(sq[:], 0.0)
            nc.gpsimd.memset(y15[:], 0.0)

            # collective warmup (absorbs first-collective latency during L0)
            wz = stpool.tile([128, 2], F32, tag="warmz")
            nc.vector.memset(wz[:], 0.0)
            nc.sync.dma_start(out=warm_in[:], in_=wz[:])
            nc.gpsimd.collective_compute(
                "AllGather", AL.bypass, replica_groups=rg,
                ins=[warm_in[:].opt()], outs=[warm_out[:].opt()])

            chunks = None   # per-rt (hi, lo) transposed bf16 chunks

            for l in range(N_LAYERS):
                wbase = 0 if l == 0 else K1T + 2 * (l - 1)
                is_upd = l in UPDATE_LAYERS

                # ---- feature matmul + Y assembly + xp hi ----
                xks = None
                for rt in range(RT):
                    b, vt = divmod(rt, NT)
                    ps = psA.tile([128, H], F32, tag="sup")
                    if l == 0:
                        if rt % 2 == 0:
                            xks = []
                            for kt in range(K1T):
                                xk = x0pool.tile([128, 256], BF, tag=f"x0_{kt}")
                                nc.sync.dma_start(
                                    out=xk[:],
                                    in_=x0t_in[kt, :, rt * 128:(rt + 2) * 128])
                                xks.append(xk)
                        for kt in range(K1T):
                            lhs = (xks[kt][:, 0:128] if rt % 2 == 0
                                   else xks[kt][:, 128:256])
                            nc.tensor.matmul(ps[:], lhsT=lhs,
                                             rhs=ws[:, kt, :],
                                             start=(kt == 0), stop=(kt == K1T - 1))
                    else:
                        hi, lo = chunks[rt]
                        nc.tensor.matmul(ps[:], lhsT=hi[:], rhs=ws[:, wbase, :],
                                         start=True, stop=False)
                        nc.tensor.matmul(ps[:], lhsT=lo[0:64, :],
                                         rhs=ws[0:64, wbase + 1, :],
                                         start=False, stop=True)
                    # Y slice: ysub = support[:, :64] * recip_norm  (bf16)
                    nc.vector.tensor_scalar(
                        out=Y[:, vt, b * SL:(b + 1) * SL], in0=ps[:, 0:SL],
                        scalar1=rn[:, vt:vt + 1], scalar2=None, op0=AL.mult)
                    # xp hi: support[:, 64:192] + b_hi (+ row-sum partial)
                    nc.vector.scalar_tensor_tensor(
                        out=xp[:, rt, 0:128], in0=ps[:, SL:H], scalar=0.0,
                        in1=bbc[:, l, :], op0=AL.add, op1=AL.add,
                        accum_out=sA[:, rt:rt + 1])

                # Y bias row (v = VP-1 -> partition 127 of vtile 20)
                nc.sync.dma_start(out=Y[127:128, NT - 1, :],
                                  in_=ybias_in[l:l + 1, :])

                # ---- adjacency matmuls + xp lo ----
                for ut in (range(NT) if PHASE >= 2 else []):
                    M = 128 if ut < NT - 1 else V - 128 * (NT - 1)
                    pside = psB.tile([128, BC * SL], F32, tag="side")
                    for vt in range(NT):
                        nc.tensor.matmul(
                            pside[0:M, :],
                            lhsT=adjT[:, vt, ut * 128:ut * 128 + M],
                            rhs=Y[:, vt, :],
                            start=(vt == 0), stop=(vt == NT - 1))
                    for b2 in range(BC):
                        rt2 = b2 * NT + ut
                        nc.vector.tensor_scalar(
                            out=xp[0:M, rt2, 128:H],
                            in0=pside[0:M, b2 * SL:(b2 + 1) * SL],
                            scalar1=0.0, scalar2=0.0, op0=AL.add, op1=AL.add,
                            accum_out=sB[0:M, rt2:rt2 + 1])

                # ---- sumsq pass ----
                for rt in (range(RT) if PHASE >= 3 else []):
                    scr = scrpool.tile([128, H], BF, tag="sqscr")
                    nc.scalar.activation(scr[:], xp[:, rt, :], AF.Square,
                                         accum_out=sq[:, rt:rt + 1])

                # ---- stats combine + AllGather + BN coefficients ----
                if PHASE < 4:
                    if debug:
                        nc.sync.dma_start(out=dbg_xp[:], in_=xp[:])
                    break
                sAB = stpool.tile([128, RT], F32, tag="sAB")
                nc.vector.tensor_tensor(out=sAB[:], in0=sA[:], in1=sB[:],
                                        op=AL.add)
                st = stpool.tile([128, NT, 2], F32, tag="st")
                nc.vector.tensor_reduce(
                    out=st[:, :, 0:1],
                    in_=sAB[:].rearrange("p (b t) -> p t b", b=BC),
                    axis=mybir.AxisListType.X, op=AL.add)
                nc.vector.tensor_reduce(
                    out=st[:, :, 1:2],
                    in_=sq[:].rearrange("p (b t) -> p t b", b=BC),
                    axis=mybir.AxisListType.X, op=AL.add)
                nc.sync.dma_start(out=ag_in[l][:], in_=st[:].rearrange("p t s -> p (t s)"))
                nc.gpsimd.collective_compute(
                    "AllGather", AL.bypass, replica_groups=rg,
                    ins=[ag_in[l][:].opt()], outs=[ag_out[l][:].opt()])
                gth = stpool.tile([128, NT * 2, NCORE], F32, tag="gth")
                nc.sync.dma_start(out=gth[:],
                                  in_=ag_out[l][:].rearrange("r p f -> p f r"))
                stg = stpool.tile([128, NT, 2], F32, tag="stg")
                nc.vector.tensor_reduce(
                    out=stg[:].rearrange("p t s -> p (t s)"), in_=gth[:],
                    axis=mybir.AxisListType.X, op=AL.add)

                mu = stpool.tile([128, NT], F32, tag="mu")
                m2 = stpool.tile([128, NT], F32, tag="m2")
                var = stpool.tile([128, NT], F32, tag="var")
                sd = stpool.tile([128, NT], F32, tag="sd")
                rv = stpool.tile([128, NT], F32, tag="rv")
                a_sb = stpool.tile([128, NT], F32, tag="a")
                d_sb = stpool.tile([128, NT], F32, tag="d")
                nc.scalar.mul(mu[:], stg[:, :, 0], 1.0 / CNT)
                nc.scalar.mul(m2[:], stg[:, :, 1], 1.0 / CNT)
                nc.vector.tensor_tensor(out=var[:], in0=mu[:], in1=mu[:],
                                        op=AL.mult)
                nc.vector.tensor_tensor(out=var[:], in0=m2[:], in1=var[:],
                                        op=AL.subtract)
                nc.scalar.activation(sd[:], var[:], AF.Sqrt,
                                     bias=eps_t[:, 0:1], scale=1.0)
                nc.vector.reciprocal(rv[:], sd[:])
                nc.vector.tensor_tensor(out=a_sb[:], in0=gb[:, l, :, 0],
                                        in1=rv[:], op=AL.mult)
                nc.vector.tensor_tensor(out=d_sb[:], in0=mu[:], in1=a_sb[:],
                                        op=AL.mult)
                nc.vector.tensor_tensor(out=d_sb[:], in0=gb[:, l, :, 1],
                                        in1=d_sb[:], op=AL.subtract)

                # ---- BN apply (+ feats update) + transposes ----
                if PHASE < 5:
                    if debug:
                        nc.sync.dma_start(out=dbg_xp[:], in_=xp[:])
                    break
                newchunks = []
                for rt in range(RT):
                    b, vt = divmod(rt, NT)
                    yb = ypool.tile([128, 256], BF, tag="yb")
                    nc.vector.memset(yb[:, H:256], 0.0)
                    if is_upd:
                        yf = fpool.tile([128, H], F32, tag="yf")
                        nc.scalar.activation(
                            yf[:], xp[:, rt, :], AF.Relu,
                            scale=a_sb[:, vt:vt + 1], bias=d_sb[:, vt:vt + 1])
                        fin = fpool.tile([128, H], F32, tag="fin")
                        src = f192_in if l == 1 else feats_out
                        nc.sync.dma_start(out=fin[:], in_=src[rt, :, :])
                        fnew = fpool.tile([128, H], F32, tag="fnew")
                        nc.vector.tensor_tensor(out=fnew[:], in0=fin[:],
                                                in1=yf[:], op=AL.add)
                        nc.sync.dma_start(out=feats_out[rt, :, :], in_=fnew[:])
                        nc.scalar.copy(yb[:, 0:H], fnew[:])
                    else:
                        nc.scalar.activation(
                            yb[:, 0:H], xp[:, rt, :], AF.Relu,
                            scale=a_sb[:, vt:vt + 1], bias=d_sb[:, vt:vt + 1])
                    if debug and l == N_LAYERS - 1:
                        nc.sync.dma_start(out=dbg_y[rt, :, :], in_=yb[:, 0:H])
                    hi = chpool.tile([128, 128], BF, tag="chi")
                    lo = chpool.tile([128, 128], BF, tag="clo")
                    nc.sync.dma_start(out=hi[:], in_=yb[:, 0:128], transpose=True)
                    nc.sync.dma_start(out=lo[:], in_=yb[:, 128:256], transpose=True)
                    newchunks.append((hi, lo))
                chunks = newchunks
                if debug and l == N_LAYERS - 1:
                    nc.sync.dma_start(out=dbg_xp[:], in_=xp[:])

            # ---- final layer (gc15 / coords) ----
            if WITH_FINAL:
                for rt in range(RT):
                    b, vt = divmod(rt, NT)
                    hi, lo = chunks[rt]
                    ps = psA.tile([128, 8], F32, tag="sup")
                    nc.tensor.matmul(ps[:], lhsT=hi[:], rhs=ws[:, 32, 0:8],
                                     start=True, stop=False)
                    nc.tensor.matmul(ps[:], lhsT=lo[0:64, :],
                                     rhs=ws[0:64, 33, 0:8],
                                     start=False, stop=True)
                    nc.vector.tensor_scalar(
                        out=y15[:, vt, b * 2:(b + 1) * 2], in0=ps[:, 0:2],
                        scalar1=rn[:, vt:vt + 1], scalar2=None, op0=AL.mult)
                    nc.scalar.activation(
                        c2sb[:, rt:rt + 1], ps[:, 2:3], AF.Identity,
                        bias=c2b_t[:, 0:1], scale=1.0)
                nc.sync.dma_start(out=y15[127:128, NT - 1, :],
                                  in_=ybias15_in[:])
                for uc in range(6):
                    wdt = min(512, V - uc * 512)
                    pst = psB.tile([2 * BC, 512], F32, tag="side")
                    for vt in range(NT):
                        nc.tensor.matmul(
                            pst[:, 0:wdt], lhsT=y15[:, vt, :],
                            rhs=adjT[:, vt, uc * 512:uc * 512 + wdt],
                            start=(vt == 0), stop=(vt == NT - 1))
                    s15t = scrpool.tile([2 * BC, 512], F32, tag="s15")
                    nc.scalar.copy(s15t[:, 0:wdt], pst[:, 0:wdt])
                    nc.sync.dma_start(
                        out=st15_out[:, uc * 512:uc * 512 + wdt],
                        in_=s15t[:, 0:wdt])
                nc.sync.dma_start(out=c2_out[:], in_=c2sb[:])

    nc.compile()
    return nc


_CACHE = {}


def _get_program(meta):
    key = (N_LAYERS, WITH_FINAL, meta["b15_2"])
    if key not in _CACHE:
        _CACHE[key] = build_program(meta)
    return _CACHE[key]


def kernel(**inputs):
    in_maps, meta = host_prep(inputs)
    nc = _get_program(meta)
    res = run_bass_kernel_spmd(nc, in_maps, core_ids=list(range(NCORE)))
    results = res.results

    feats = np.empty((B, V, H), np.float32)
    coords = np.empty((B, V, OUT), np.float32)
    scale = 2.0 ** -(KIN_FINAL)
    for c in range(NCORE):
        raw = np.asarray(results[c]["feats_raw"])     # [RT, 128, H]
        fr = raw.reshape(BC, VP, H)[:, :V] * scale
        feats[c * BC:(c + 1) * BC] = fr
        c2 = np.asarray(results[c]["c2"])             # [128, RT]
        cc = c2.T.reshape(BC, NT, 128).reshape(BC, VP)[:, :V]
        coords[c * BC:(c + 1) * BC, :, 0] = cc
        s15 = np.asarray(results[c]["st15"])          # [8, VP]
        s15 = s15.reshape(BC, 2, VP)[:, :, :V]
        coords[c * BC:(c + 1) * BC, :, 1] = s15[:, 0]
        coords[c * BC:(c + 1) * BC, :, 2] = s15[:, 1]
    return feats, coords
